# revision 8
# baseline (speedup 1.0000x reference)
"""Trainium2 Bass kernel for the patch-retrieval module (nn_DMB_46737834115118).

Sharding: 8 cores = 4 batch samples x 2 image-row halves. Each core:
  - projects its half of hs (128ch -> 3ch) on the PE in fp32,
  - computes the full-sample depthwise conv / gradient / diff on DVE+GPSIMD+ACT,
  - builds patch matrices, runs sim = hp @ mp^T (fp32 PE), softmax (ACT exp),
    argmax (DVE max_index), soft = P @ mp (fp32 PE), and the hard gather
    (indirect DMA from a DRAM patch table),
  - writes its (12, 256, 512) slice of the output.
Host only slices inputs and concatenates the 8 output slices.
"""
import numpy as np

import concourse.bass as bass
import concourse.tile as tile
from concourse import bacc, mybir
from concourse.bass_utils import run_bass_kernel_spmd
from concourse.masks import make_identity

F32 = mybir.dt.float32
U32 = mybir.dt.uint32
AX = mybir.AxisListType
OP = mybir.AluOpType
ACTF = mybir.ActivationFunctionType

P = 128          # partitions
HALF = 256       # image rows per core
W = 512          # image cols
NPT = 16         # patch size
N1L = 16         # local patch rows per core (256/16)
N2 = 32          # patch cols
NLOC = 512       # local patches per core
NFULL = 1024     # full-sample patches
D = 768          # 3*16*16
DH = 2304        # 9*16*16


def _ap(base, off, dims):
    """Custom access pattern on the same tensor as `base` (offset in elements)."""
    return bass.AP(base.tensor, base.offset + off, [list(d) for d in dims])


def _build():
    nc = bacc.Bacc("TRN2", target_bir_lowering=False, debug=False, num_devices=8)

    hs_d = nc.dram_tensor("hs", [P, HALF, W], F32, kind="ExternalInput").ap()
    ms_d = nc.dram_tensor("msf", [3, W, W], F32, kind="ExternalInput").ap()
    spec_d = nc.dram_tensor("spec", [P, 3], F32, kind="ExternalInput").ap()
    wcv_d = nc.dram_tensor("wcv", [P, 27], F32, kind="ExternalInput").ap()
    out_d = nc.dram_tensor("out", [12, HALF, W], F32, kind="ExternalOutput").ap()

    with tile.TileContext(nc) as tc:
        with tc.tile_pool(name="const", bufs=1) as constp, \
             tc.tile_pool(name="msio", bufs=2) as msio, \
             tc.tile_pool(name="convw", bufs=2) as convw, \
             tc.tile_pool(name="hsp", bufs=2) as hsp, \
             tc.tile_pool(name="stgp", bufs=2) as stgp, \
             tc.tile_pool(name="persist", bufs=1) as pers, \
             tc.tile_pool(name="stream", bufs=2) as strm, \
             tc.tile_pool(name="psum", bufs=2, space="PSUM") as psp, \
             tc.tile_pool(name="dram", bufs=1, space="DRAM") as drp:

            spec_s = constp.tile([P, 3], F32, tag="spec")
            wcv_s = constp.tile([P, 27], F32, tag="wcv")
            ident = constp.tile([P, P], F32, tag="ident")
            eps_s = constp.tile([P, 1], F32, tag="eps")
            nc.vector.memset(eps_s[:], 1e-6)
            nc.scalar.dma_start(spec_s[:], spec_d[:])
            nc.scalar.dma_start(wcv_s[:], wcv_d[:])
            make_identity(nc, ident[:])

            hm = drp.tile([NFULL, DH], F32, tag="hmat")
            msdimg = drp.tile([3, W, W], F32, tag="msdimg")

            # persistent SBUF tensors
            mp_t = [pers.tile([P, D], F32, tag=f"mp{k}", name=f"mp{k}") for k in range(8)]
            mpT_t = [pers.tile([P, NFULL], F32, tag=f"mpT{k}", name=f"mpT{k}") for k in range(6)]
            hp_t = [pers.tile([P, D], F32, tag=f"hp{k}", name=f"hp{k}") for k in range(4)]
            hpT_t = [pers.tile([P, NLOC], F32, tag=f"hpT{k}", name=f"hpT{k}") for k in range(6)]
            pT_t = [pers.tile([P, NLOC], F32, tag=f"pT{k}", name=f"pT{k}") for k in range(8)]
            recip_t = [pers.tile([P, 1], F32, tag=f"rc{k}", name=f"rc{k}") for k in range(4)]
            idx_t = [pers.tile([P, 8], U32, tag=f"ix{k}", name=f"ix{k}") for k in range(4)]

            # ============ Phase 1: hs projection -> hp tiles ============
            # 256 rows; groups of 16 rows -> one (128, 2048) psum generation
            for g in range(16):
                pp = psp.tile([P, 2048], F32, tag="mm")
                for i in range(16):
                    r = 16 * g + i          # local image row
                    if r % 8 == 0:
                        hs_sb = hsp.tile([P, 4096], F32, tag="hs")
                        nc.sync.dma_start(
                            hs_sb[:], hs_d[:, r:r + 8, :])
                    j, s = i % 4, i // 4
                    nc.tensor.matmul(
                        pp[32 * j:32 * j + 3, 512 * s:512 * s + 512],
                        spec_s[:],
                        hs_sb[:, 512 * (r % 8):512 * (r % 8) + 512],
                        start=True, stop=True, tile_position=(0, 32 * j))
                # evacuate to staging with free-dim permute:
                # psum free = s*512 + n2*16 + pj ; staging free = n2*64 + s*16 + pj
                st = stgp.tile([P, 2048], F32, tag="stg")
                for j in range(4):
                    lo = 32 * j
                    pin = _ap(pp[:], 2048 * lo,
                              [[2048, 3], [512, 4], [16, 32], [1, 16]])
                    pout = _ap(st[:], 2048 * lo,
                               [[2048, 3], [16, 4], [64, 32], [1, 16]])
                    if j < 2:
                        nc.vector.tensor_copy(pout, pin)
                    else:
                        nc.scalar.copy(pout, pin)
                # staging -> hp tile (partition = local patch index)
                # hp tile k2 = g//4, partitions 32*(g%4).. ; free = c*256+pi*16+pj
                k2, pq = g // 4, g % 4
                hp = hp_t[k2]
                for j in range(4):
                    for c in range(3):
                        src = _ap(st[:], 2048 * (32 * j + c),
                                  [[2048, 1], [64, 32], [16, 4], [1, 16]])
                        dst = _ap(hp[:], D * 32 * pq + 256 * c + 16 * j,
                                  [[D, 32], [64, 4], [1, 16]])
                        nc.scalar.dma_start(dst, src)

            # hp -> hpT via PE transpose
            for k2 in range(4):
                for kc in range(6):
                    tp = psp.tile([P, 2048], F32, tag="mm")
                    nc.tensor.transpose(
                        tp[:, 0:P], hp_t[k2][:, P * kc:P * kc + P], ident[:])
                    nc.vector.tensor_copy(
                        hpT_t[kc][:, P * k2:P * k2 + P], tp[:, 0:P])

            # ============ Phase 2: conv features on full sample ============
            for c in range(3):
                for t in range(4):
                    r0 = 128 * t
                    ctr = msio.tile([P, 514], F32, tag="ctr")
                    up = msio.tile([P, 514], F32, tag="up")
                    dn = msio.tile([P, 514], F32, tag="dn")
                    for tl in (ctr, up, dn):
                        nc.vector.memset(tl[:, 0:1], 0.0)
                        nc.vector.memset(tl[:, 513:514], 0.0)
                    nc.scalar.dma_start(ctr[:, 1:513], ms_d[c, r0:r0 + 128, :])
                    if t == 0:
                        nc.vector.memset(up[0:1, :], 0.0)
                        nc.scalar.dma_start(up[1:128, 1:513],
                                            ms_d[c, 0:127, :])
                    else:
                        nc.scalar.dma_start(up[:, 1:513],
                                            ms_d[c, r0 - 1:r0 + 127, :])
                    if t == 3:
                        nc.vector.memset(dn[96:128, :], 0.0)
                        nc.scalar.dma_start(dn[0:127, 1:513],
                                            ms_d[c, r0 + 1:r0 + 128, :])
                    else:
                        nc.scalar.dma_start(dn[:, 1:513],
                                            ms_d[c, r0 + 1:r0 + 129, :])

                    # depthwise 3x3 (9 FMAs on DVE)
                    msd = convw.tile([P, 512], F32, tag="msd")
                    srcs = (up, ctr, dn)
                    first = True
                    for di in range(3):
                        for dj in range(3):
                            shv = srcs[di][:, dj:dj + 512]
                            wap = wcv_s[:, 9 * c + 3 * di + dj:
                                        9 * c + 3 * di + dj + 1]
                            if first:
                                nc.vector.tensor_scalar_mul(msd[:], shv, wap)
                                first = False
                            else:
                                nc.vector.scalar_tensor_tensor(
                                    msd[:], shv, wap, msd[:],
                                    op0=OP.mult, op1=OP.add)
                    # gradient magnitude + ms_diff
                    gv = convw.tile([P, 512], F32, tag="gv")
                    gh = convw.tile([P, 512], F32, tag="gh")
                    gv2 = convw.tile([P, 512], F32, tag="gv2")
                    dif = convw.tile([P, 512], F32, tag="dif")
                    nc.gpsimd.tensor_tensor(gv[:], dn[:, 1:513], up[:, 1:513],
                                            op=OP.subtract)
                    nc.gpsimd.tensor_tensor(gh[:], ctr[:, 2:514],
                                            ctr[:, 0:512], op=OP.subtract)
                    nc.scalar.square(gv2[:], gv[:])
                    nc.gpsimd.tensor_tensor(gh[:], gh[:], gh[:], op=OP.mult)
                    nc.gpsimd.tensor_tensor(gv2[:], gv2[:], gh[:], op=OP.add)
                    nc.scalar.activation(gv[:], gv2[:], ACTF.Sqrt,
                                         bias=eps_s[:, 0:1])
                    nc.gpsimd.tensor_tensor(dif[:], ctr[:, 1:513], msd[:],
                                            op=OP.subtract)

                    # msd -> DRAM image (for mp loads)
                    nc.scalar.dma_start(msdimg[c, r0:r0 + 128, :], msd[:])
                    # hmat writes: channels c (ms), 3+c (grad), 6+c (diff)
                    hmb = hm[:]
                    for (ch, tl, ap_src) in (
                            (c, ctr, ctr[:, 1:513]),
                            (3 + c, gv, gv[:]),
                            (6 + c, dif, dif[:])):
                        for q in range(8):          # rows 16q..16q+16 of tile
                            n1 = 8 * t + q
                            src = _ap(ap_src, ap_src.ap[0][0] * 16 * q,
                                      [[ap_src.ap[0][0], 16], [16, 32],
                                       [1, 16]])
                            dst = _ap(hmb, DH * 32 * n1 + 256 * ch,
                                      [[16, 16], [DH, 32], [1, 16]])
                            nc.sync.dma_start(dst, src)

            # mp tiles from msd image (full sample)
            for k in range(8):
                for c in range(3):
                    for q in range(4):
                        n1 = 4 * k + q
                        src = _ap(msdimg[:], W * W * c + W * NPT * n1 + 0,
                                  [[16, 32], [512, 16], [1, 16]])
                        dst = _ap(mp_t[k][:], D * 32 * q + 256 * c,
                                  [[D, 32], [16, 16], [1, 16]])
                        nc.scalar.dma_start(dst, src)
            # mp -> mpT
            for k in range(8):
                for kc in range(6):
                    tp = psp.tile([P, 2048], F32, tag="mm")
                    nc.tensor.transpose(
                        tp[:, 0:P], mp_t[k][:, P * kc:P * kc + P], ident[:])
                    nc.vector.tensor_copy(
                        mpT_t[kc][:, P * k:P * k + P], tp[:, 0:P])

            # ============ Phase 3: sim, softmax, argmax, soft, gather ======
            for mt in range(4):
                sm = psp.tile([P, 2048], F32, tag="mm")
                for kc in range(6):
                    for hlf in range(2):
                        nc.tensor.matmul(
                            sm[:, 512 * hlf:512 * hlf + 512],
                            hpT_t[kc][:, P * mt:P * mt + P],
                            mpT_t[kc][:, 512 * hlf:512 * hlf + 512],
                            start=(kc == 0), stop=(kc == 5))
                negmax = strm.tile([P, 1], F32, tag="ngm")
                nc.vector.tensor_reduce(negmax[:], sm[:, 0:1024], axis=AX.X,
                                        op=OP.max, negate=True)
                mx8 = strm.tile([P, 8], F32, tag="mx8")
                nc.vector.max(mx8[:], sm[:, 0:1024])
                nc.vector.max_index(idx_t[mt][:], mx8[:], sm[:, 0:1024])
                pt = strm.tile([P, 1024], F32, tag="P")
                rowsum = strm.tile([P, 1], F32, tag="rs")
                nc.scalar.activation(pt[:], sm[:, 0:1024], ACTF.Exp,
                                     bias=negmax[:, 0:1],
                                     accum_out=rowsum[:, 0:1])
                nc.vector.reciprocal(recip_t[mt][:], rowsum[:])
                # P^T blocks
                for mc in range(8):
                    tp = psp.tile([P, 2048], F32, tag="mm")
                    nc.tensor.transpose(
                        tp[:, 0:P], pt[:, P * mc:P * mc + P], ident[:])
                    nc.vector.tensor_copy(
                        pT_t[mc][:, P * mt:P * mt + P], tp[:, 0:P])

            for mt in range(4):
                sf = psp.tile([P, 2048], F32, tag="mm")
                for mc in range(8):
                    for (off, wd) in ((0, 512), (512, 256)):
                        nc.tensor.matmul(
                            sf[:, off:off + wd],
                            pT_t[mc][:, P * mt:P * mt + P],
                            mp_t[mc][:, off:off + wd],
                            start=(mc == 0), stop=(mc == 7))
                soft = strm.tile([P, D], F32, tag="soft")
                nc.scalar.mul(soft[:], sf[:, 0:D], recip_t[mt][:, 0:1])
                # soft -> out channels 0..2
                for c in range(3):
                    for q in range(4):
                        n1 = 4 * mt + q
                        src = _ap(soft[:], D * 32 * q + 256 * c,
                                  [[D, 32], [1, 256]])
                        dst = _ap(out_d[:], HALF * W * c + W * NPT * n1,
                                  [[16, 32], [512, 16], [1, 16]])
                        nc.scalar.dma_start(dst, src)

                gath = strm.tile([P, DH], F32, tag="gath")
                nc.gpsimd.indirect_dma_start(
                    out=gath[:], out_offset=None, in_=hm[:],
                    in_offset=bass.IndirectOffsetOnAxis(
                        ap=idx_t[mt][:, 0:1], axis=0))
                for ch in range(9):
                    for q in range(4):
                        n1 = 4 * mt + q
                        src = _ap(gath[:], DH * 32 * q + 256 * ch,
                                  [[DH, 32], [1, 256]])
                        dst = _ap(out_d[:], HALF * W * (3 + ch) + W * NPT * n1,
                                  [[16, 32], [512, 16], [1, 16]])
                        nc.sync.dma_start(dst, src)

    nc.compile()
    return nc


_NC = None


def _get_nc():
    global _NC
    if _NC is None:
        _NC = _build()
    return _NC


def kernel(hs, ms, spectral_matrix, kernel_weight):
    hs = np.asarray(hs, dtype=np.float32)
    ms = np.asarray(ms, dtype=np.float32)
    spec = np.ascontiguousarray(np.asarray(spectral_matrix, dtype=np.float32))
    kw = np.asarray(kernel_weight, dtype=np.float32)
    wcv = np.ascontiguousarray(
        np.broadcast_to(kw.reshape(1, 27), (P, 27))).astype(np.float32)

    nc = _get_nc()
    in_maps = []
    for core in range(8):
        b, h = core // 2, core % 2
        in_maps.append({
            "hs": np.ascontiguousarray(hs[b, :, HALF * h:HALF * (h + 1), :]),
            "msf": np.ascontiguousarray(ms[b]),
            "spec": spec,
            "wcv": wcv,
        })
    res = run_bass_kernel_spmd(nc, in_maps, list(range(8)))
    out = np.empty((4, 12, 512, 512), np.float32)
    for core in range(8):
        b, h = core // 2, core % 2
        out[b, :, HALF * h:HALF * (h + 1), :] = res.results[core]["out"]
    return out


# revision 35
# speedup vs baseline: 20491.7263x; 20491.7263x over previous
"""Trainium2 Bass kernel for the patch-retrieval module (nn_DMB_46737834115118).

Sharding: 8 cores = 4 batch samples x 2 image-row halves. Each core:
  - projects its half of hs (128ch -> 3ch) on the PE in fp32,
  - computes the full-sample depthwise conv / gradient / diff on DVE+GPSIMD+ACT,
  - builds patch matrices, runs sim = hp @ mp^T (fp32 PE), softmax (ACT exp),
    argmax (DVE max_index), soft = P @ mp (fp32 PE), and the hard gather
    (indirect DMA from a DRAM patch table),
  - writes its (12, 256, 512) slice of the output.
Host only slices inputs and concatenates the 8 output slices.
"""
import numpy as np

import concourse.bass as bass
import concourse.tile as tile
from concourse import bacc, mybir
from concourse.bass_utils import run_bass_kernel_spmd
from concourse.masks import make_identity

F32 = mybir.dt.float32
U32 = mybir.dt.uint32
AX = mybir.AxisListType
OP = mybir.AluOpType
ACTF = mybir.ActivationFunctionType

P = 128          # partitions
HALF = 256       # image rows per core
W = 512          # image cols
NPT = 16         # patch size
N1L = 16         # local patch rows per core (256/16)
N2 = 32          # patch cols
NLOC = 512       # local patches per core
NFULL = 1024     # full-sample patches
D = 768          # 3*16*16
DH = 2304        # 9*16*16


def _ap(base, off, dims):
    """Custom access pattern on the same tensor as `base` (offset in elements)."""
    return bass.AP(base.tensor, base.offset + off, [list(d) for d in dims])


def _build(skip_hmat=False, skip_hp_dma=False, skip_out=False, skip_mp=False, skip_proj_mm=False, pool_every=55):
    nc = bacc.Bacc("TRN2", target_bir_lowering=False, debug=False, num_devices=8)
    _ctr = [0]

    def pick(hwdge_eng):
        _ctr[0] += 1
        if pool_every and (_ctr[0] * pool_every) % 100 < pool_every:
            return nc.gpsimd
        return hwdge_eng

    hs_d = nc.dram_tensor("hs", [P, HALF, W], F32, kind="ExternalInput").ap()
    ms_d = nc.dram_tensor("msf", [3, W, W], F32, kind="ExternalInput").ap()
    spec_d = nc.dram_tensor("spec", [P, 3], F32, kind="ExternalInput").ap()
    wcv_d = nc.dram_tensor("wcv", [P, 27], F32, kind="ExternalInput").ap()
    msp_d = nc.dram_tensor("msp", [NFULL, D], F32, kind="ExternalInput").ap()
    out_d = nc.dram_tensor("out", [12, HALF, W], F32, kind="ExternalOutput").ap()

    with tile.TileContext(nc) as tc:
        with tc.tile_pool(name="const", bufs=1) as constp, \
             tc.tile_pool(name="msio", bufs=2) as msio, \
             tc.tile_pool(name="convw", bufs=2) as convw, \
             tc.tile_pool(name="hsp", bufs=2) as hsp, \
             tc.tile_pool(name="stgp", bufs=2) as stgp, \
             tc.tile_pool(name="persist", bufs=1) as pers, \
             tc.tile_pool(name="stream", bufs=2) as strm, \
             tc.tile_pool(name="psum", bufs=2, space="PSUM") as psp, \
             tc.tile_pool(name="dram", bufs=1, space="DRAM") as drp:

            spec_s = constp.tile([P, 3], F32, tag="spec")
            wcv_s = constp.tile([P, 27], F32, tag="wcv")
            ident = constp.tile([P, P], F32, tag="ident")
            eps_s = constp.tile([P, 1], F32, tag="eps")
            nc.vector.memset(eps_s[:], 1e-6)
            nc.scalar.dma_start(spec_s[:], spec_d[:])
            nc.scalar.dma_start(wcv_s[:], wcv_d[:])
            make_identity(nc, ident[:])

            hm = drp.tile([NFULL, D], F32, tag="hmat")
            msdp = drp.tile([NFULL, D], F32, tag="msdp")
            msdimg = drp.tile([3, W, W], F32, tag="msdimg")

            # persistent SBUF tensors
            mp_t = [pers.tile([P, D], F32, tag=f"mp{k}", name=f"mp{k}") for k in range(8)]
            mpT_t = [pers.tile([P, NFULL], F32, tag=f"mpT{k}", name=f"mpT{k}") for k in range(6)]
            hp_t = [pers.tile([P, D], F32, tag=f"hp{k}", name=f"hp{k}") for k in range(4)]
            hpT_t = [pers.tile([P, NLOC], F32, tag=f"hpT{k}", name=f"hpT{k}") for k in range(6)]
            pT_t = [pers.tile([P, NLOC], F32, tag=f"pT{k}", name=f"pT{k}") for k in range(8)]
            recip_t = [pers.tile([P, 1], F32, tag=f"rc{k}", name=f"rc{k}") for k in range(4)]
            idx_t = [pers.tile([P, 8], U32, tag=f"ix{k}", name=f"ix{k}") for k in range(4)]

            # ============ Phase 2: conv features (emitted interleaved) ====
            def emit_conv(c, t):
                    r0 = 128 * t
                    ctr = msio.tile([P, 514], F32, tag="ctr")
                    up = msio.tile([P, 514], F32, tag="up")
                    dn = msio.tile([P, 514], F32, tag="dn")
                    for tl in (ctr, up, dn):
                        nc.vector.memset(tl[:, 0:1], 0.0)
                        nc.vector.memset(tl[:, 513:514], 0.0)
                    nc.scalar.dma_start(ctr[:, 1:513], ms_d[c, r0:r0 + 128, :])
                    if t == 0:
                        nc.vector.memset(up[0:1, :], 0.0)
                        nc.scalar.dma_start(up[1:128, 1:513],
                                            ms_d[c, 0:127, :])
                    else:
                        nc.scalar.dma_start(up[:, 1:513],
                                            ms_d[c, r0 - 1:r0 + 127, :])
                    if t == 3:
                        nc.vector.memset(dn[96:128, :], 0.0)
                        nc.scalar.dma_start(dn[0:127, 1:513],
                                            ms_d[c, r0 + 1:r0 + 128, :])
                    else:
                        nc.scalar.dma_start(dn[:, 1:513],
                                            ms_d[c, r0 + 1:r0 + 129, :])

                    # depthwise 3x3 (9 FMAs on DVE)
                    msd = convw.tile([P, 512], F32, tag="msd")
                    srcs = (up, ctr, dn)
                    first = True
                    for di in range(3):
                        for dj in range(3):
                            shv = srcs[di][:, dj:dj + 512]
                            wap = wcv_s[:, 9 * c + 3 * di + dj:
                                        9 * c + 3 * di + dj + 1]
                            if first:
                                nc.vector.tensor_scalar_mul(msd[:], shv, wap)
                                first = False
                            else:
                                nc.vector.scalar_tensor_tensor(
                                    msd[:], shv, wap, msd[:],
                                    op0=OP.mult, op1=OP.add)
                    # gradient magnitude + ms_diff
                    gv = convw.tile([P, 512], F32, tag="gv")
                    gh = convw.tile([P, 512], F32, tag="gh")
                    gv2 = convw.tile([P, 512], F32, tag="gv2")
                    nc.vector.tensor_tensor(gv[:], dn[:, 1:513], up[:, 1:513],
                                            op=OP.subtract)
                    nc.vector.tensor_tensor(gh[:], ctr[:, 2:514],
                                            ctr[:, 0:512], op=OP.subtract)
                    nc.scalar.square(gv2[:], gv[:])
                    nc.vector.tensor_tensor(gh[:], gh[:], gh[:], op=OP.mult)
                    nc.vector.tensor_tensor(gv2[:], gv2[:], gh[:], op=OP.add)
                    nc.scalar.activation(gv[:], gv2[:], ACTF.Sqrt,
                                         bias=eps_s[:, 0:1])

                    # msd -> DRAM image (for mp loads)
                    nc.scalar.dma_start(msdimg[c, r0:r0 + 128, :], msd[:])
                    # hmat writes: channels c (ms), 3+c (grad), 6+c (diff)
                    hmb = hm[:]
                    ap_src = gv[:]
                    for q in range(8):          # rows 16q..16q+16 of tile
                        if skip_hmat:
                            continue
                        n1 = 8 * t + q
                        src = _ap(ap_src, ap_src.ap[0][0] * 16 * q,
                                  [[ap_src.ap[0][0], 16], [16, 32],
                                   [1, 16]])
                        dst = _ap(hmb, D * 32 * n1 + 256 * c,
                                  [[16, 16], [D, 32], [1, 16]])
                        pick(nc.scalar).dma_start(dst, src)


            def emit_mp(k, c):
                for q in range(4):
                    if skip_mp:
                        continue
                    n1 = 4 * k + q
                    src = _ap(msdimg[:], W * W * c + W * NPT * n1 + 0,
                              [[16, 32], [512, 16], [1, 16]])
                    dst = _ap(mp_t[k][:], D * 32 * q + 256 * c,
                              [[D, 32], [16, 16], [1, 16]])
                    pick(nc.scalar).dma_start(dst, src)

            def emit_mpT(k):
                for kc in range(6):
                    tp = psp.tile([P, 2048], F32, tag="mm", name="tpm")
                    nc.tensor.transpose(
                        tp[:, 0:P], mp_t[k][:, P * kc:P * kc + P], ident[:])
                    nc.vector.tensor_copy(
                        mpT_t[kc][:, P * k:P * k + P], tp[:, 0:P])


            def emit_hpT(k2):
                for kc in range(6):
                    tp = psp.tile([P, 2048], F32, tag="mm", name="tph")
                    nc.tensor.transpose(
                        tp[:, 0:P], hp_t[k2][:, P * kc:P * kc + P], ident[:])
                    nc.vector.tensor_copy(
                        hpT_t[kc][:, P * k2:P * k2 + P], tp[:, 0:P])


            # ============ Phase 1: hs projection -> hp tiles ============
            # 256 rows; groups of 16 rows -> one (128, 2048) psum generation
            for g in range(16):
                pp = psp.tile([P, 2048], F32, tag="mm")
                for i in range(16):
                    r = 16 * g + i          # local image row
                    if r % 8 == 0:
                        hs_sb = hsp.tile([P, 4096], F32, tag="hs")
                        nc.sync.dma_start(
                            hs_sb[:], hs_d[:, r:r + 8, :])
                    j, s = i % 4, i // 4
                    nc.tensor.matmul(
                        pp[32 * j:32 * j + 3, 512 * s:512 * s + 512],
                        spec_s[:],
                        hs_sb[:, 512 * (r % 8):512 * (r % 8) + 512],
                        start=True, stop=True, tile_position=(0, 32 * j))
                # evacuate to staging with free-dim permute:
                # psum free = s*512 + n2*16 + pj ; staging free = n2*64 + s*16 + pj
                st = stgp.tile([P, 2048], F32, tag="stg")
                for j in range(4):
                    lo = 32 * j
                    pin = _ap(pp[:], 2048 * lo,
                              [[2048, 3], [512, 4], [16, 32], [1, 16]])
                    pout = _ap(st[:], 2048 * lo,
                               [[2048, 3], [16, 4], [64, 32], [1, 16]])
                    if j < 2:
                        nc.vector.tensor_copy(pout, pin)
                    else:
                        nc.scalar.copy(pout, pin)
                # staging -> hp tile (partition = local patch index)
                k2, pq = g // 4, g % 4
                hp = hp_t[k2]
                for j in range(4):
                    for c in range(3):
                        if skip_hp_dma:
                            continue
                        src = _ap(st[:], 2048 * (32 * j + c),
                                  [[2048, 1], [64, 32], [16, 4], [1, 16]])
                        dst = _ap(hp[:], D * 32 * pq + 256 * c + 16 * j,
                                  [[D, 32], [64, 4], [1, 16]])
                        pick(nc.scalar).dma_start(dst, src)

            for _k2 in range(4):
                emit_hpT(_k2)
            for _t in range(4):
                for _c in range(3):
                    emit_conv(_c, _t)
                    emit_mp(2 * _t, _c)
                    emit_mp(2 * _t + 1, _c)
            for _k in range(8):
                emit_mpT(_k)
                pick(nc.sync).dma_start(msdp[P * _k:P * (_k + 1), :],
                                        mp_t[_k][:])

            # ============ Phase 3: sim, softmax, argmax, soft, gather ======
            for mt in range(4):
                sm = psp.tile([P, 2048], F32, tag="mm")
                for kc in range(6):
                    for hlf in range(2):
                        nc.tensor.matmul(
                            sm[:, 512 * hlf:512 * hlf + 512],
                            hpT_t[kc][:, P * mt:P * mt + P],
                            mpT_t[kc][:, 512 * hlf:512 * hlf + 512],
                            start=(kc == 0), stop=(kc == 5))
                negmax = strm.tile([P, 1], F32, tag="ngm")
                nc.vector.tensor_reduce(negmax[:], sm[:, 0:1024], axis=AX.X,
                                        op=OP.max, negate=True)
                mx8 = strm.tile([P, 8], F32, tag="mx8")
                nc.vector.max(mx8[:], sm[:, 0:1024])
                nc.vector.max_index(idx_t[mt][:], mx8[:], sm[:, 0:1024])
                pt = strm.tile([P, 1024], F32, tag="P")
                rowsum = strm.tile([P, 1], F32, tag="rs")
                nc.scalar.activation(pt[:], sm[:, 0:1024], ACTF.Exp,
                                     bias=negmax[:, 0:1],
                                     accum_out=rowsum[:, 0:1])
                nc.vector.reciprocal(recip_t[mt][:], rowsum[:])
                # P^T blocks
                for mc in range(8):
                    tp = psp.tile([P, 2048], F32, tag="mm")
                    nc.tensor.transpose(
                        tp[:, 0:P], pt[:, P * mc:P * mc + P], ident[:])
                    nc.vector.tensor_copy(
                        pT_t[mc][:, P * mt:P * mt + P], tp[:, 0:P])

            for mt in range(4):
                sf = psp.tile([P, 2048], F32, tag="mm")
                for mc in range(8):
                    for (off, wd) in ((0, 512), (512, 256)):
                        nc.tensor.matmul(
                            sf[:, off:off + wd],
                            pT_t[mc][:, P * mt:P * mt + P],
                            mp_t[mc][:, off:off + wd],
                            start=(mc == 0), stop=(mc == 7))
                soft = strm.tile([P, D], F32, tag="soft")
                nc.scalar.mul(soft[:], sf[:, 0:D], recip_t[mt][:, 0:1])
                # soft -> out channels 0..2
                for c in range(3):
                    for q in range(4):
                        if skip_out:
                            continue
                        n1 = 4 * mt + q
                        src = _ap(soft[:], D * 32 * q + 256 * c,
                                  [[D, 32], [1, 256]])
                        dst = _ap(out_d[:], HALF * W * c + W * NPT * n1,
                                  [[16, 32], [512, 16], [1, 16]])
                        pick(nc.scalar).dma_start(dst, src)

                gmsp = strm.tile([P, D], F32, tag="gmsp")
                nc.gpsimd.indirect_dma_start(
                    out=gmsp[:], out_offset=None, in_=msp_d[:],
                    in_offset=bass.IndirectOffsetOnAxis(
                        ap=idx_t[mt][:, 0:1], axis=0))
                gath = strm.tile([P, D], F32, tag="gath")
                nc.gpsimd.indirect_dma_start(
                    out=gath[:], out_offset=None, in_=hm[:],
                    in_offset=bass.IndirectOffsetOnAxis(
                        ap=idx_t[mt][:, 0:1], axis=0))
                gmsd = strm.tile([P, D], F32, tag="gmsd")
                nc.gpsimd.indirect_dma_start(
                    out=gmsd[:], out_offset=None, in_=msdp[:],
                    in_offset=bass.IndirectOffsetOnAxis(
                        ap=idx_t[mt][:, 0:1], axis=0))
                # gathered ms_diff = gathered ms - gathered ms_d
                nc.vector.tensor_tensor(gmsd[:], gmsp[:], gmsd[:],
                                        op=OP.subtract)
                for ch in range(9):
                    for q in range(4):
                        if skip_out:
                            continue
                        n1 = 4 * mt + q
                        if ch < 3:
                            src = _ap(gmsp[:], D * 32 * q + 256 * ch,
                                      [[D, 32], [1, 256]])
                        elif ch < 6:
                            src = _ap(gath[:], D * 32 * q + 256 * (ch - 3),
                                      [[D, 32], [1, 256]])
                        else:
                            src = _ap(gmsd[:], D * 32 * q + 256 * (ch - 6),
                                      [[D, 32], [1, 256]])
                        dst = _ap(out_d[:], HALF * W * (3 + ch) + W * NPT * n1,
                                  [[16, 32], [512, 16], [1, 16]])
                        pick(nc.sync).dma_start(dst, src)

    nc.compile()
    return nc


_NC = None


def _get_nc():
    global _NC
    if _NC is None:
        _NC = _build()
    return _NC


def kernel(hs, ms, spectral_matrix, kernel_weight):
    hs = np.asarray(hs, dtype=np.float32)
    ms = np.asarray(ms, dtype=np.float32)
    spec = np.ascontiguousarray(np.asarray(spectral_matrix, dtype=np.float32))
    kw = np.asarray(kernel_weight, dtype=np.float32)
    wcv = np.ascontiguousarray(
        np.broadcast_to(kw.reshape(1, 27), (P, 27))).astype(np.float32)

    nc = _get_nc()
    in_maps = []
    for core in range(8):
        b, h = core // 2, core % 2
        msp = np.ascontiguousarray(
            ms[b].reshape(3, 32, 16, 32, 16).transpose(1, 3, 0, 2, 4)
            .reshape(NFULL, D))
        in_maps.append({
            "hs": np.ascontiguousarray(hs[b, :, HALF * h:HALF * (h + 1), :]),
            "msf": np.ascontiguousarray(ms[b]),
            "msp": msp,
            "spec": spec,
            "wcv": wcv,
        })
    res = run_bass_kernel_spmd(nc, in_maps, list(range(8)))
    out = np.empty((4, 12, 512, 512), np.float32)
    for core in range(8):
        b, h = core // 2, core % 2
        out[b, :, HALF * h:HALF * (h + 1), :] = res.results[core]["out"]
    return out
